# revision 99
# baseline (speedup 1.0000x reference)
"""Trainium2 Bass kernel: multi-head attention (B=4, N=2048, D=768, 12 heads).

Sharding: 8 cores = 4 batches x 2 head-groups (6 heads each).
Each core computes, for its (batch, head-group):
    qT/kT = (W[:,cols].T @ x.T)   in [64*2, N] stacked head pairs
    sT    = k q^T (scaled, exp'd) -> attn^T tiles [keys, queries], 2-kt score
            groups in a 3-deep psum ring; exp split ScalarE/DVE (Schraudolph)
    av    = e^T @ [v|1]           -> flipped attn@v: e is the stationary lhsT
            and v streams, so each pass costs 65 output columns instead of
            512; both heads share one psum bank via its byte-granular
            pending-zero state
    o     = av / sums             per-partition scalars (sums ride as col 64)
    oT    = dma-transpose(o)      [q, d] -> [d, q] via the XBAR
    yT    = Wp[rows,:].T @ oT     partial output [768, N] in bf16
Host sums the two partial yT per batch (the all-reduce of the row-split Wp
projection) and adds bp.

Schedule: software-pipelined over 12 (qb, pair) iterations - while iteration
k runs its scores/exp groups, the PE fills the exp-chain slack with
iteration k-1's AV chains plus projection / output-projection chunks placed
in per-group hook slots.
"""

import numpy as np
import ml_dtypes

B, N, DIM = 4, 2048, 768
HEADS, HD = 12, 64
SCALE = HD ** -0.5
NCORES = 8
HLOC = HEADS // 2        # heads per core
PAIRS = HLOC // 2        # head pairs per core
P = 128
QB = 512                 # query block
NQB = N // QB            # 4
KT = N // P              # 16 key tiles
KTG = 2                  # key tiles per exp group (psum tile = KTG banks)
GROUPS = [(2 * i, 2) for i in range(8)]
KC = DIM // P            # 6 contraction chunks for projections
VPAD = 72                # padded per-head v row
# Schraudolph bf16 exp on DVE for a subset of score tiles: balances the exp
# work between ScalarE and VectorE. (group_idx, head_idx) pairs handled by DVE.
DVE_EXP = frozenset({(1, 0), (2, 1), (3, 0), (5, 1), (6, 0), (7, 1)})
SCHRAU_A = 128.0 / float(np.log(2.0))      # bf16 exponent scale
SCHRAU_B = 16256.0 - 7.4                   # 127<<7 minus centering constant

_cache = {}
EPOOL_BUFS = 22
NPOOL_BUFS = 10


def _build(dump=False):
    import concourse.bacc as bacc
    import concourse.mybir as mybir
    import concourse.tile as tile
    from concourse._compat import get_trn_type

    fp32 = mybir.dt.float32
    bf16 = mybir.dt.bfloat16
    Exp = mybir.ActivationFunctionType.Exp
    mult = mybir.AluOpType.mult

    nc = bacc.Bacc(
        get_trn_type() or "TRN2",
        target_bir_lowering=False,
        debug=False,
        enable_asserts=False,
        num_devices=NCORES,
    )

    xT = nc.dram_tensor("xT", [DIM, N], bf16, kind="ExternalInput").ap()
    wq = nc.dram_tensor("wq", [DIM, HLOC * HD], bf16, kind="ExternalInput").ap()
    wk = nc.dram_tensor("wk", [DIM, HLOC * HD], bf16, kind="ExternalInput").ap()
    wv = nc.dram_tensor("wv", [DIM, HLOC * HD], bf16, kind="ExternalInput").ap()
    wp = nc.dram_tensor("wp", [HLOC * HD, DIM], bf16, kind="ExternalInput").ap()
    yT = nc.dram_tensor("yT", [DIM, N], bf16, kind="ExternalOutput").ap()
    if dump:
        dbg_qT = nc.dram_tensor("dbg_qT", [PAIRS, P, N], bf16, kind="ExternalOutput").ap()
        dbg_kT = nc.dram_tensor("dbg_kT", [PAIRS, P, N], bf16, kind="ExternalOutput").ap()
        dbg_v = nc.dram_tensor("dbg_v", [P, KT, HLOC, VPAD], bf16, kind="ExternalOutput").ap()
        dbg_oT = nc.dram_tensor("dbg_oT", [P, PAIRS, N], bf16, kind="ExternalOutput").ap()
        dbg_av = nc.dram_tensor("dbg_av", [P, QB], fp32, kind="ExternalOutput").ap()

    with tile.TileContext(nc) as tc:
        with (
            tc.tile_pool(name="const", bufs=1) as cpool,
            tc.tile_pool(name="exp", bufs=EPOOL_BUFS) as epool,
            tc.tile_pool(name="norm", bufs=NPOOL_BUFS) as npool,
            tc.tile_pool(name="qkp", bufs=3, space="PSUM") as qkp,
            tc.tile_pool(name="avp", bufs=2, space="PSUM") as avp,
        ):
            # ---------------- input loads ----------------
            # kc-interleaved so the first projection chain starts as soon as
            # its first contraction chunks land
            wq_sb = cpool.tile([P, KC, HLOC * HD], bf16, name="wq_sb")
            wqr = wq.rearrange("(o p) m -> p o m", p=P)
            xT_sb = cpool.tile([P, KC, N], bf16, name="xT_sb")
            xTr = xT.rearrange("(o p) n -> p o n", p=P)
            wk_sb = cpool.tile([P, KC, HLOC * HD], bf16, name="wk_sb")
            wkr = wk.rearrange("(o p) m -> p o m", p=P)
            nc.sync.dma_start(wq_sb[:, 0:3, 0:P], wqr[:, 0:3, 0:P])
            nc.sync.dma_start(xT_sb[:, 0:3, 0:QB], xTr[:, 0:3, 0:QB])
            nc.sync.dma_start(wq_sb[:, 3:6, 0:P], wqr[:, 3:6, 0:P])
            nc.sync.dma_start(xT_sb[:, 3:6, 0:QB], xTr[:, 3:6, 0:QB])
            nc.sync.dma_start(xT_sb[:, 0:3, QB:2 * QB], xTr[:, 0:3, QB:2 * QB])
            nc.sync.dma_start(wk_sb[:, 0:3, 0:P], wkr[:, 0:3, 0:P])
            nc.sync.dma_start(xT_sb[:, 3:6, QB:2 * QB], xTr[:, 3:6, QB:2 * QB])
            nc.sync.dma_start(wk_sb[:, 3:6, 0:P], wkr[:, 3:6, 0:P])
            for t in range(2, NQB):
                ts_ = slice(t * QB, (t + 1) * QB)
                nc.sync.dma_start(xT_sb[:, 0:3, ts_], xTr[:, 0:3, ts_])
                nc.sync.dma_start(xT_sb[:, 3:6, ts_], xTr[:, 3:6, ts_])
            wv_sb = cpool.tile([P, KC, HLOC * HD], bf16, name="wv_sb")
            nc.sync.dma_start(wv_sb, wv.rearrange("(o p) m -> p o m", p=P))
            nc.sync.dma_start(wq_sb[:, :, P:], wqr[:, :, P:])
            nc.sync.dma_start(wk_sb[:, :, P:], wkr[:, :, P:])
            wp_sb = cpool.tile([P, PAIRS, DIM], bf16, name="wp_sb")
            nc.sync.dma_start(wp_sb, wp.rearrange("(o p) m -> p o m", p=P))

            # HAM warm-up: dummy matmuls fill the startup DMA wait so the
            # PE clock-gate is already released when the projections start
            warm = cpool.tile([P, QB], bf16, name="warm")
            nc.gpsimd.memset(warm, 0.0)
            wps = qkp.tile([P, KTG * QB], fp32, name="wps", tag="qk")
            for _w in range(8):
                nc.tensor.matmul(wps[:, 0:QB], lhsT=warm[:, 0:P], rhs=warm)
            wps = qkp.tile([P, KTG * QB], fp32, name="wps", tag="qk")
            for _w in range(8):
                nc.tensor.matmul(wps[:, 0:QB], lhsT=warm[:, 0:P], rhs=warm)

            qT_sb = [cpool.tile([P, N], bf16, name=f"qT{pr}") for pr in range(PAIRS)]
            kT_sb = [cpool.tile([P, N], bf16, name=f"kT{pr}") for pr in range(PAIRS)]
            # v with a trailing ones column per head: [P, kt, head, 64+1]
            v_sb = cpool.tile([P, KT, HLOC, VPAD], bf16, name="v_sb")
            oT_sb = cpool.tile([P, PAIRS, N], bf16, name="oT_sb")
            ebias = cpool.tile([P, 1], fp32, name="ebias")
            nc.vector.memset(ebias, 0.0)
            nc.vector.memset(v_sb[:, :, :, HD], 1.0)
            ident = cpool.tile([P, P], bf16, name="ident")
            from concourse.masks import make_identity
            make_identity(nc, ident)

            # ---------------- emission helpers ----------------
            def emit_projqk_group(pair, wi, half, nbs=(0, 1)):
                    w_sb, dst = ((wq_sb, qT_sb[pair]), (wk_sb, kT_sb[pair]))[wi]
                    ps = qkp.tile([P, KTG * QB], fp32, name="ps_qk", tag="qk")
                    for nb in nbs:
                        col = half * 2 * QB + nb * QB
                        for kc in range(KC):
                            nc.tensor.matmul(
                                ps[:, nb * QB:(nb + 1) * QB],
                                lhsT=w_sb[:, kc, pair * P:(pair + 1) * P],
                                rhs=xT_sb[:, kc, col:col + QB],
                                start=(kc == 0),
                                stop=(kc == KC - 1),
                            )
                        nc.vector.tensor_copy(
                            out=dst[:, (half * 2 + nb) * QB:(half * 2 + nb + 1) * QB],
                            in_=ps[:, nb * QB:(nb + 1) * QB],
                        )

            def emit_projqk(pair, order=((0, 0), (0, 1), (1, 0), (1, 1))):
                # order = sequence of (which-of-q/k, token-half); for pair0
                # q-half0 and k-half0 come first so qb0 attention starts early
                for wi, half in order:
                    emit_projqk_group(pair, wi, half)

            def emit_projv_group(g):
                    ps = qkp.tile([P, KTG * QB], fp32, name="ps_v", tag="qk")
                    for j in range(2):
                        nt = g * 2 + j
                        for kc in range(KC):
                            nc.tensor.matmul(
                                ps[:, j * QB: j * QB + HLOC * HD],
                                lhsT=xT_sb[:, kc, nt * P:(nt + 1) * P],
                                rhs=wv_sb[:, kc, :],
                                start=(kc == 0),
                                stop=(kc == KC - 1),
                            )
                        nc.scalar.copy(
                            v_sb[:, nt, :, 0:HD],
                            ps[:, j * QB: j * QB + HLOC * HD].rearrange(
                                "p (h d) -> p h d", d=HD
                            ),
                        )

            def emit_av_chain_steps(qb, pair, etiles, gi, qsubs, av_tiles):
                # flipped attn@v accumulation steps for exp group gi:
                # out[q, d] += e[k, q]^T @ v[k, d|1]; e is the stationary
                # operand so each pass streams only 65 output columns.
                # Head A's chain owns the psum zero-region start/stop; head
                # B's chain rides the same bank on the byte-granular
                # pending-zero state (its first write replaces, then
                # accumulates), so both heads share one [128, 130] region.
                g0, glen = GROUPS[gi]
                for qsub in qsubs:
                    av = av_tiles[qsub]
                    for idx in range(2):
                        e = etiles[gi][idx]
                        col0 = idx * (HD + 1)
                        for j in range(glen):
                            kt = g0 + j
                            if idx == 0:
                                flags = dict(start=(kt == 0), stop=(kt == KT - 1))
                            else:
                                flags = dict(start=False, stop=False,
                                             skip_group_check=True)
                            nc.tensor.matmul(
                                av[:, col0:col0 + HD + 1],
                                lhsT=e[:, j * QB + qsub * P: j * QB + (qsub + 1) * P],
                                rhs=v_sb[:, kt, 2 * pair + idx, 0:HD + 1],
                                **flags,
                            )

            def emit_av_norm(qb, pair, qsub, av, pe_transpose=False):
                # normalise by the softmax sums (cols HD and 2*HD+1 of av),
                # pack both heads into [128 q, 128 d], transpose to oT via
                # the DMA XBAR (or the PE for latency-critical tail qsubs).
                rec = npool.tile([P, 2], fp32, name="rec", tag="rec")
                nc.vector.reciprocal_approx_fast(out=rec[:, 0:1], in_=av[:, HD:HD + 1])
                nc.vector.reciprocal_approx_fast(
                    out=rec[:, 1:2], in_=av[:, 2 * HD + 1:2 * HD + 2])
                onorm = npool.tile([P, P], bf16, name="onorm", tag="onorm")
                # per-partition scale on ScalarE (GPSIMD cannot touch PSUM)
                Copy = mybir.ActivationFunctionType.Copy
                nc.scalar.activation(
                    onorm[:, 0:HD], av[:, 0:HD], Copy, scale=rec[:, 0:1])
                nc.scalar.activation(
                    onorm[:, HD:P], av[:, HD + 1:2 * HD + 1], Copy,
                    scale=rec[:, 1:2])
                if dump and qb == 0 and pair == 0 and qsub == 0:
                    nc.sync.dma_start(dbg_av, av)
                qs0 = qb * QB + qsub * P
                if pe_transpose:
                    tr = avp.tile([P, P], bf16, name="tr", tag="av")
                    nc.tensor.transpose(tr, onorm, ident)
                    nc.vector.tensor_copy(
                        out=oT_sb[:, pair, qs0:qs0 + P], in_=tr)
                else:
                    nc.sync.dma_start(
                        oT_sb[:, pair, qs0:qs0 + P], onorm, transpose=True)

            def emit_outproj(qb, ms=None, alt_evac=False, cols=None,
                             psum_pool=None):
                c0, c1 = (0, QB) if cols is None else cols
                qs = slice(qb * QB + c0, qb * QB + c1)
                w = c1 - c0
                for m in (range(DIM // P) if ms is None else ms):
                    pool = psum_pool if psum_pool is not None else avp
                    yps = pool.tile([P, QB], fp32, name="yps",
                                    tag="av" if pool is avp else "qk")
                    for kc in range(PAIRS):
                        nc.tensor.matmul(
                            yps[:, 0:w],
                            lhsT=wp_sb[:, kc, m * P:(m + 1) * P],
                            rhs=oT_sb[:, kc, qs],
                            start=(kc == 0),
                            stop=(kc == PAIRS - 1),
                        )
                    ysb = npool.tile([P, QB], bf16, name="ysb", tag="ysb")
                    if alt_evac and m % 2 == 0:
                        nc.scalar.copy(ysb[:, 0:w], yps[:, 0:w])
                    else:
                        nc.vector.tensor_copy(out=ysb[:, 0:w], in_=yps[:, 0:w])
                    # terminal block: spread output DMAs over both HWDGE queues
                    dma_eng = nc.scalar if (alt_evac and m % 2 == 1) else nc.sync
                    dma_eng.dma_start(yT[m * P:(m + 1) * P, qs], ysb[:, 0:w])

            # ---------------- schedule ----------------
            # Software-pipelined: during iteration k's scores/exp groups, the
            # PE fills the exp wait slots with the PREVIOUS iteration's AV
            # chains (all exps complete -> no wait-queue head-of-line
            # blocking), plus projection / output-projection chunks.
            def emit_attention_groups(qb, pair, hooks, dve_exp=DVE_EXP):
                """scores+exp for (qb, pair); hooks[gi] (if present) is
                emitted right after group gi's exp. Returns etiles."""
                etiles = []
                qs = slice(qb * QB, (qb + 1) * QB)
                for g0, glen in GROUPS:
                    psA = qkp.tile([P, KTG * QB], fp32, name="psA", tag="qk")[:, :glen * QB]
                    psB = qkp.tile([P, KTG * QB], fp32, name="psB", tag="qk")[:, :glen * QB]
                    gi_ = len(etiles)
                    # emit the half that feeds the slower (DVE) exp first so
                    # its psum completes earlier
                    b_first = False
                    for j in range(glen):
                        kt = g0 + j
                        ks = slice(kt * P, (kt + 1) * P)
                        halves = (
                            (psB, HD, P, (HD, 0)), (psA, 0, HD, (0, 0)),
                        ) if b_first else (
                            (psA, 0, HD, (0, 0)), (psB, HD, P, (HD, 0)),
                        )
                        for ps_, r0, r1, tp in halves:
                            nc.tensor.matmul(
                                ps_[:, j * QB:(j + 1) * QB],
                                lhsT=kT_sb[pair][r0:r1, ks],
                                rhs=qT_sb[pair][r0:r1, qs],
                                tile_position=tp,
                            )
                    eA = epool.tile([P, KTG * QB], bf16, name="eA", tag="eA")[:, :glen * QB]
                    eB = epool.tile([P, KTG * QB], bf16, name="eB", tag="eB")[:, :glen * QB]
                    gi = len(etiles)
                    for idx, (e, ps) in enumerate(((eA, psA), (eB, psB))):
                        if (gi, idx) in dve_exp:
                            nc.vector.tensor_scalar(
                                e.bitcast(mybir.dt.int16), ps,
                                SCALE * SCHRAU_A, SCHRAU_B,
                                mybir.AluOpType.mult, mybir.AluOpType.add,
                            )
                        else:
                            nc.scalar.activation(e, ps, Exp, scale=SCALE,
                                                 bias=ebias[:, :])
                    etiles.append((eA, eB))
                    if gi < len(hooks) and hooks[gi] is not None:
                        hooks[gi]()
                return etiles

            def emit_av_qsub(qb, pair, etiles, qsub, pe_transpose=False,
                             psum_pool=None):
                # one full flipped-AV chain (both heads) + norm + transpose
                pool = psum_pool if psum_pool is not None else avp
                av = pool.tile([P, QB], fp32, name="avq",
                               tag="av" if pool is avp else "qk")
                for gi in range(len(GROUPS)):
                    emit_av_chain_steps(qb, pair, etiles, gi, (qsub,), {qsub: av})
                emit_av_norm(qb, pair, qsub, av, pe_transpose=pe_transpose)

            def av_item(qb, pair, etiles, qsub, pe_tr=False):
                return lambda: emit_av_qsub(qb, pair, etiles, qsub,
                                            pe_transpose=pe_tr)

            # iteration order: (qb, pair) pairs; (1,0) early so pair0's
            # qT/kT reuse is hot and projqk(1/2) can trail in hook slots
            ITERS = [(0, 0), (1, 0), (0, 1), (0, 2), (1, 1), (1, 2),
                     (2, 0), (2, 1), (2, 2), (3, 0), (3, 1), (3, 2)]

            # pair0 q-half1 is first read at (2,0) = iteration 6 (keys span
            # all tokens, queries don't); it rides iteration 3's free hook
            # slot instead of gating iteration 0's scores here
            emit_projqk(0, order=((0, 0), (1, 0), (1, 1)))

            prev = None          # (qb, pair, etiles) of previous iteration
            outproj_chunks = [
                (qb, m) for qb in range(NQB - 1) for m in range(DIM // P)
            ]
            op_i = {"i": 0}

            def outproj_item():
                def run():
                    if op_i["i"] < len(outproj_chunks):
                        qb_, m_ = outproj_chunks[op_i["i"]]
                        op_i["i"] += 1
                        emit_outproj(qb_, ms=(m_,))
                return run

            for it, (qb, pair) in enumerate(ITERS):
                items = []
                if prev is not None:
                    pqb, ppair, pet = prev
                    items = [av_item(pqb, ppair, pet, q) for q in range(4)]
                # extra-work slots
                if it == 0:
                    items = [lambda g=g: emit_projv_group(g) for g in range(6)]
                elif it == 1:
                    items += [lambda: emit_projqk_group(1, 0, 0),
                              lambda: emit_projqk_group(1, 1, 0)]
                elif it == 2:
                    items += [lambda: emit_projqk_group(2, 0, 0),
                              lambda: emit_projqk_group(2, 1, 0)]

                elif (qb, pair) in ((1, 1), (1, 2), (2, 0), (2, 1), (2, 2),
                                    (3, 0), (3, 1), (3, 2)):
                    items += [outproj_item(), outproj_item()]
                # bias the fill toward the late groups so the iteration
                # boundary (ring handoff to the next pair's scores) stays
                # covered with independent PE work
                hooks = [None] * len(GROUPS)
                slots = (0, 1, 2, 4, 5, 7)[:len(items)] if len(items) <= 6 \
                    else list(range(len(GROUPS)))
                for s, f in zip(slots, items):
                    hooks[s] = f
                dve_set = DVE_EXP if it >= 2 else (DVE_EXP | {(0, 1), (4, 1), (7, 0)})
                etiles = emit_attention_groups(qb, pair, hooks, dve_exp=dve_set)
                # post-group work that must complete before the next
                # iteration's scores can use it
                if it == 0:
                    emit_projv_group(6)
                    emit_projv_group(7)
                elif it == 1:
                    emit_projqk_group(1, 1, 1)
                elif it == 2:
                    emit_projqk_group(2, 1, 1)
                elif it == 4:
                    emit_projqk_group(1, 0, 1, nbs=(0,))
                elif it == 5:
                    emit_projqk_group(0, 0, 1, nbs=(0,))
                    emit_projqk_group(2, 0, 1, nbs=(0,))
                elif it == 6:
                    emit_projqk_group(0, 0, 1, nbs=(1,))
                elif it == 7:
                    emit_projqk_group(1, 0, 1, nbs=(1,))
                elif it == 8:
                    emit_projqk_group(2, 0, 1, nbs=(1,))
                prev = (qb, pair, etiles)

            # tail: the scores psum banks are free now, so the last AV
            # chains and outproj blocks borrow them (no av-ring waits);
            # leftover outproj chunks interleave between chains to cover
            # norm latency, and the last two qsubs transpose on the PE.
            pqb, ppair, pet = prev

            def tail_op():
                if op_i["i"] < len(outproj_chunks):
                    qb_, m_ = outproj_chunks[op_i["i"]]
                    op_i["i"] += 1
                    emit_outproj(qb_, ms=(m_,), alt_evac=True, psum_pool=qkp)

            emit_av_qsub(pqb, ppair, pet, 0, pe_transpose=True, psum_pool=qkp)
            emit_av_qsub(pqb, ppair, pet, 1, pe_transpose=True, psum_pool=qkp)
            emit_av_qsub(pqb, ppair, pet, 2, pe_transpose=True, psum_pool=qkp)
            emit_av_qsub(pqb, ppair, pet, 3, pe_transpose=True, psum_pool=qkp)
            while op_i["i"] < len(outproj_chunks):
                tail_op()
            if dump:
                nc.sync.dma_start(dbg_oT, oT_sb)
            emit_outproj(NQB - 1, alt_evac=True, psum_pool=qkp)

    nc.compile()
    return nc


def _get_nc():
    nc = _cache.get("nc")
    if nc is None:
        nc = _build()
        _cache["nc"] = nc
    return nc


def make_in_maps(x, Wq, Wk, Wv, Wp):
    bf = ml_dtypes.bfloat16
    x = np.asarray(x, np.float32)
    Wq = np.asarray(Wq, np.float32)
    Wk = np.asarray(Wk, np.float32)
    Wv = np.asarray(Wv, np.float32)
    Wp = np.asarray(Wp, np.float32)
    xTs = [np.ascontiguousarray(x[b].T).astype(bf) for b in range(B)]
    in_maps = []
    for c in range(NCORES):
        b, hg = divmod(c, 2)
        cs = slice(hg * HLOC * HD, (hg + 1) * HLOC * HD)
        in_maps.append(
            {
                "xT": xTs[b],
                "wq": np.ascontiguousarray(Wq[:, cs]).astype(bf),
                "wk": np.ascontiguousarray(Wk[:, cs]).astype(bf),
                "wv": np.ascontiguousarray(Wv[:, cs]).astype(bf),
                "wp": np.ascontiguousarray(Wp[cs, :]).astype(bf),
            }
        )
    return in_maps


def assemble(outs, bp):
    bp32 = np.asarray(bp, np.float32)
    y = np.empty((B, N, DIM), np.float32)
    for b in range(B):
        y[b] = (np.asarray(outs[2 * b]["yT"], np.float32)
                + np.asarray(outs[2 * b + 1]["yT"], np.float32)).T + bp32
    return y


def kernel(x, Wq, Wk, Wv, Wp, bp):
    from concourse.bass_utils import run_bass_kernel_spmd

    nc = _get_nc()
    in_maps = make_in_maps(x, Wq, Wk, Wv, Wp)
    res = run_bass_kernel_spmd(nc, in_maps, core_ids=list(range(NCORES)))
    _cache["last_result"] = res
    return assemble(res.results, bp)
